# revision 28
# baseline (speedup 1.0000x reference)
"""Trainium2 Bass kernel for a masked-attention block (MAB).

Computation (per batch element):
    Q = X@Wq + bq ; K = Y@Wk + bk ; V = Y@Wv + bv
    logits = per-head Qh@Kh^T / 32, masked keys -> -inf, softmax over keys
    attn   = A @ Vh (concat heads)
    O1 = LN(Q + attn; g1,b1)
    O  = LN(O1 + relu(O1@Wo + bo); g2,b2)

Sharding: pure data-parallel, one batch element per NeuronCore (B=8 = 8 cores).

On-device dataflow is "feature-major": activations live in SBUF transposed
([model_dim -> 8x128 partitions, token -> free]) so every matmul chains
without transposes.  Softmax denominators and LayerNorm stats are
partition-dim reductions done with all-ones stationary matmuls (which also
broadcast the result across partitions for free).

Schedule (engines are in-order, so program order = execution order):
  * warm-up matmuls at t=0 (HAM un-throttle) while Wq/X DMAs stream in
    need-order.
  * Q proj (bf16), K proj (fp8 DoubleRow), then immediately the head-0/1
    logits so the ScalarE exp stream -- the attention-phase roofline at
    ~9.2us/head -- starts as early as possible.  The V projection's fp8-DR
    matmuls are interleaved into the PE slack underneath the exp stream
    (V block ng=0 before head 0 needs it, ng=1 spread across heads 1-2).
  * softmax denominator + AV in fp8-DR; zt = pa*rc + bv + qt (bv folded
    into the residual: softmax rows sum to 1, so A@(V + 1 bv^T) = A@V+bv).
  * LN1 -> Wo -> LN2 tail in two 512-token chunks, program-ordered so one
    chunk's DVE work overlaps the other chunk's PE work; LN temps are all
    bf16 (packed 2x DVE mode), squares/means on ScalarE, the Wo residual
    adds on GpSimd, so no single engine gates the tail.
"""

import math
import numpy as np

import concourse.bass as bass
import concourse.mybir as mybir
import concourse.tile as tile
from concourse import bacc
from concourse.bass_utils import run_bass_kernel_spmd

P = 128
NX = 1024
NY = 1024
DIM = 1024
H = 8
KO = DIM // P          # 8 partition sub-tiles of the model dim
QC = 512
NQC = NX // QC         # 2
F32 = mybir.dt.float32
BF16 = mybir.dt.bfloat16
FP8 = mybir.dt.float8e4
AF = mybir.ActivationFunctionType
ALU = mybir.AluOpType
DR = mybir.MatmulPerfMode.DoubleRow
SCALE = 1.0 / 32.0     # 1/sqrt(DIM)
EPS = 1e-5
USE_FP8 = True
W_PRESCALE = 32.0      # host multiplies Wk/Wv by this when USE_FP8
N_WARMUP = 24          # warm-up matmuls at t=0 (HAM un-throttle + DMA overlap)


def _build():
    nc = bacc.Bacc("TRN2", target_bir_lowering=False, debug=False,
                   enable_asserts=False)
    kvdt = FP8 if USE_FP8 else BF16
    kv_scale = (1.0 / W_PRESCALE) if USE_FP8 else 1.0

    # ---- DRAM I/O (per-core shapes) ----
    XT = nc.dram_tensor("XT", [DIM, NX], BF16, kind="ExternalInput").ap()
    YT = nc.dram_tensor("YT", [DIM, NY], kvdt, kind="ExternalInput").ap()
    MB = nc.dram_tensor("MB", [NY], F32, kind="ExternalInput").ap()
    WQ = nc.dram_tensor("Wq", [DIM, DIM], BF16, kind="ExternalInput").ap()
    WK = nc.dram_tensor("Wk", [DIM, DIM], kvdt, kind="ExternalInput").ap()
    WV = nc.dram_tensor("Wv", [DIM, DIM], kvdt, kind="ExternalInput").ap()
    WO = nc.dram_tensor("Wo", [DIM, DIM], BF16, kind="ExternalInput").ap()
    Vecs = {}
    for vname in ("bq", "bk", "bv", "bo", "g1", "b1", "g2", "b2"):
        Vecs[vname] = nc.dram_tensor(vname, [DIM], F32, kind="ExternalInput").ap()
    OT = nc.dram_tensor("OT", [DIM, NX], BF16, kind="ExternalOutput").ap()

    xt3 = XT.rearrange("(ko p) q -> p ko q", p=P)
    yt3 = YT.rearrange("(ko p) q -> p ko q", p=P)
    wq3 = WQ.rearrange("(ko p) d -> p ko d", p=P)
    wk3 = WK.rearrange("(ko p) d -> p ko d", p=P)
    wv3 = WV.rearrange("(ko p) d -> p ko d", p=P)
    wo3 = WO.rearrange("(ko p) d -> p ko d", p=P)
    ot3 = OT.rearrange("(do p) q -> p do q", p=P)

    with tile.TileContext(nc) as tc:
        with tc.tile_pool(name="const", bufs=1) as const, \
             tc.tile_pool(name="act", bufs=1) as actp:

            # ---- constants ----
            ones_bf = const.tile([P, P], BF16, tag="onesbf", name="ones_bf")
            nc.vector.memset(ones_bf, 1.0)
            if USE_FP8:
                ones_f8 = const.tile([P, 2, P], FP8, tag="ones8", name="ones_f8")
                nc.vector.memset(ones_f8, 1.0)
            warm_rhs = const.tile([P, QC], BF16, tag="warm", name="warm_rhs")
            nc.vector.memset(warm_rhs, 0.0)
            eps_sb = const.tile([P, 1], F32, tag="eps", name="eps_sb")
            nc.vector.memset(eps_sb, EPS)

            def vec_pko(name):
                t = const.tile([P, KO], F32, tag=f"v_{name}", name=f"{name}_sb")
                nc.sync.dma_start(t, Vecs[name].rearrange("(ko p) -> p ko", p=P))
                return t

            mb_sb = const.tile([P, KO], F32, tag="v_mb", name="mb_sb")
            nc.sync.dma_start(mb_sb, MB.rearrange("(ko p) -> p ko", p=P))
            bq_sb = vec_pko("bq")
            bk_sb = vec_pko("bk")
            bv_sb = vec_pko("bv")
            bo_sb = vec_pko("bo")
            g1_sb = vec_pko("g1")
            b1_sb = vec_pko("b1")
            g2_sb = vec_pko("g2")
            b2_sb = vec_pko("b2")

            # ---- persistent feature-major activation tiles ----
            qt = actp.tile([P, KO, NX], BF16, tag="qt", name="qt")
            ktm = actp.tile([P, KO, NY], BF16, tag="ktm", name="ktm")
            vm = actp.tile([P, KO, DIM], kvdt, tag="vm", name="vm")
            zt = actp.tile([P, KO, NX], BF16, tag="zt", name="zt")
            sq1 = actp.tile([P, KO, NX], BF16, tag="sq1", name="sq1")
            o1t = actp.tile([P, KO, NX], BF16, tag="o1t", name="o1t")
            z2t = actp.tile([P, KO, NX], BF16, tag="z2t", name="z2t")
            wo_sb = actp.tile([P, KO, DIM], BF16, tag="wo", name="wo_sb")

            # ============ Phase 1+2: projections + attention ============
            # One flat PSUM layout (16KB/partition exactly):
            #   pool A  tag "ps"  [P,NX] f32 x2  (8KB): Q/K psums, then logits
            #   pool B  tag "psv" [P,QC] f32 x2  (4KB): warm-up, V psums
            #   pool C  tags "rl"+"av" [P,QC] x1 each (4KB): denom + AV
            with tc.tile_pool(name="io", bufs=1) as iop, \
                 tc.tile_pool(name="pA", bufs=2, space="PSUM") as pA, \
                 tc.tile_pool(name="pB", bufs=2, space="PSUM") as pB, \
                 tc.tile_pool(name="pC", bufs=1, space="PSUM") as pC, \
                 tc.tile_pool(name="exp", bufs=3) as ep, \
                 tc.tile_pool(name="rcp", bufs=2) as rp:

                # ---- PE warm-up: no input deps, keeps PE busy from t=0 ----
                for i in range(N_WARMUP):
                    wp = pB.tile([P, QC], F32, tag="psv", name=f"warm{i}")
                    nc.tensor.matmul(wp, lhsT=ones_bf, rhs=warm_rhs,
                                     start=True, stop=True)

                yt = iop.tile([P, KO, NY], kvdt, tag="yt", name="yt")
                wk_sb = iop.tile([P, KO, DIM], kvdt, tag="wk", name="wk_sb")
                wv_sb = iop.tile([P, KO, DIM], kvdt, tag="wv", name="wv_sb")

                with tc.tile_pool(name="ioq", bufs=1) as ioq:
                    xt = ioq.tile([P, KO, NX], BF16, tag="xt", name="xt")
                    wq_sb = ioq.tile([P, KO, DIM], BF16, tag="wq",
                                     name="wq_sb")

                    # DMA issue order = need order.
                    for k in range(KO):
                        nc.sync.dma_start(wq_sb[:, k, 0:QC], wq3[:, k, 0:QC])
                        nc.sync.dma_start(xt[:, k, :], xt3[:, k, :])
                    for k in range(KO):
                        nc.sync.dma_start(wq_sb[:, k, QC:DIM],
                                          wq3[:, k, QC:DIM])
                    for k in range(KO):
                        nc.sync.dma_start(yt[:, k, :], yt3[:, k, :])
                    for k in range(KO):
                        nc.sync.dma_start(wk_sb[:, k, :], wk3[:, k, :])
                    for k in range(KO):
                        nc.sync.dma_start(wv_sb[:, k, :], wv3[:, k, :])
                    for k in range(KO):
                        nc.sync.dma_start(wo_sb[:, k, :], wo3[:, k, :])

                    # --- Q projection (bf16) ---
                    for do in range(KO):
                        ps = pA.tile([P, NX], F32, tag="ps", name=f"ps_q{do}")
                        for k in range(KO):
                            for qc in range(NQC):
                                qs = slice(qc * QC, (qc + 1) * QC)
                                nc.tensor.matmul(
                                    ps[:, qs],
                                    lhsT=wq_sb[:, k, do * P:(do + 1) * P],
                                    rhs=xt[:, k, qs],
                                    start=(k == 0), stop=(k == KO - 1))
                        nc.scalar.activation(
                            qt[:, do, :], ps, AF.Identity,
                            bias=bq_sb[:, do:do + 1], scale=1.0)

                # --- K projection (fp8 DoubleRow) ---
                for do in range(KO):
                    ps = pA.tile([P, NX], F32, tag="ps", name=f"ps_k{do}")
                    if USE_FP8:
                        for kp in range(KO // 2):
                            ks = slice(2 * kp, 2 * kp + 2)
                            for qc in range(NQC):
                                qs = slice(qc * QC, (qc + 1) * QC)
                                nc.tensor.matmul(
                                    ps[:, qs],
                                    lhsT=wk_sb[:, ks, do * P:(do + 1) * P],
                                    rhs=yt[:, ks, qs],
                                    start=(kp == 0), stop=(kp == KO // 2 - 1),
                                    perf_mode=DR)
                    else:
                        for k in range(KO):
                            for qc in range(NQC):
                                qs = slice(qc * QC, (qc + 1) * QC)
                                nc.tensor.matmul(
                                    ps[:, qs],
                                    lhsT=wk_sb[:, k, do * P:(do + 1) * P],
                                    rhs=yt[:, k, qs],
                                    start=(k == 0), stop=(k == KO - 1))
                    # drain on DVE (keeps ACT free for the exp stream)
                    nc.vector.tensor_scalar(
                        ktm[:, do, :], ps,
                        scalar1=kv_scale,
                        scalar2=bk_sb[:, do:do + 1],
                        op0=ALU.mult, op1=ALU.add)

                # --- PE filler queue: V-proj / denom / AV matmuls that run
                # in the PE slack underneath the ACT exp stream.  Each
                # filler emits one matmul (possibly with trailing DVE ops).
                from collections import deque
                fillers = deque()

                def add_v_fillers(ng, yo_list):
                    ns = slice(ng * QC, (ng + 1) * QC)
                    for yo in yo_list:
                        ps = [None]

                        def mk(kp, yo=yo, ps=ps, ns=ns):
                            def emit():
                                if kp == 0:
                                    ps[0] = pB.tile(
                                        [P, QC], F32, tag="psv",
                                        name=f"ps_v{ns.start}{yo}")
                                ks = slice(2 * kp, 2 * kp + 2)
                                nc.tensor.matmul(
                                    ps[0],
                                    lhsT=yt[:, ks, yo * P:(yo + 1) * P],
                                    rhs=wv_sb[:, ks, ns],
                                    start=(kp == 0), stop=(kp == KO // 2 - 1),
                                    perf_mode=DR)
                                if kp == KO // 2 - 1:
                                    nc.vector.tensor_scalar_mul(
                                        vm[:, yo, ns], ps[0], kv_scale)
                            return emit

                        for kp in range(KO // 2):
                            fillers.append(mk(kp))

                def add_dav_fillers(h, et):
                    # pr (denominator) for both chunks first, then pa (AV):
                    # pa's lhsT needs the V drains, pr doesn't.
                    rcs = {}

                    def mk_pr(qc, kp, st={}):
                        def emit():
                            if kp == 0:
                                st[qc] = pC.tile([P, QC], F32, tag="rl",
                                                 name=f"pr{h}{qc}")
                            qs = slice(qc * QC, (qc + 1) * QC)
                            ks = slice(2 * kp, 2 * kp + 2)
                            nc.tensor.matmul(
                                st[qc], lhsT=ones_f8, rhs=et[:, ks, qs],
                                start=(kp == 0), stop=(kp == KO // 2 - 1),
                                perf_mode=DR)
                            if kp == KO // 2 - 1:
                                rc = rp.tile([P, QC], F32, tag="rc",
                                             name=f"rc{h}{qc}")
                                nc.vector.reciprocal_approx_fast(rc, st[qc])
                                rcs[qc] = rc
                        return emit

                    def mk_pa(qc, kp, st={}):
                        def emit():
                            if kp == 0:
                                st[qc] = pC.tile([P, QC], F32, tag="av",
                                                 name=f"pa{h}{qc}")
                            qs = slice(qc * QC, (qc + 1) * QC)
                            ks = slice(2 * kp, 2 * kp + 2)
                            nc.tensor.matmul(
                                st[qc],
                                lhsT=vm[:, ks, h * P:(h + 1) * P],
                                rhs=et[:, ks, qs],
                                start=(kp == 0), stop=(kp == KO // 2 - 1),
                                perf_mode=DR)
                            if kp == KO // 2 - 1:
                                nc.vector.tensor_mul(zt[:, h, qs], st[qc],
                                                     rcs[qc])
                                # zt = (attn + bv) + qt  -- the bv fold
                                nc.vector.scalar_tensor_tensor(
                                    zt[:, h, qs], zt[:, h, qs],
                                    bv_sb[:, h:h + 1],
                                    qt[:, h, qs], op0=ALU.add, op1=ALU.add)
                                # LN1 square, pre-computed during attention
                                nc.vector.tensor_mul(sq1[:, h, qs],
                                                     zt[:, h, qs],
                                                     zt[:, h, qs])
                        return emit

                    for qc in range(NQC):
                        for kp in range(KO // 2):
                            fillers.append(mk_pr(qc, kp))
                    for qc in range(NQC):
                        for kp in range(KO // 2):
                            fillers.append(mk_pa(qc, kp))

                def logits_exp_slots(h, et):
                    # one exp-sized slot per kt: 2 logits matmuls + up to 3
                    # filler matmuls (fits under the 1147ns exp op)
                    for kt in range(KO):
                        pl = pA.tile([P, NX], F32, tag="ps", name=f"pl{h}{kt}")
                        for qc in range(NQC):
                            qs = slice(qc * QC, (qc + 1) * QC)
                            nc.tensor.matmul(
                                pl[:, qs],
                                lhsT=ktm[:, h, kt * P:(kt + 1) * P],
                                rhs=qt[:, h, qs],
                                start=True, stop=True)
                        nc.scalar.activation(
                            et[:, kt, :], pl, AF.Exp,
                            bias=mb_sb[:, kt:kt + 1], scale=SCALE)
                        for _ in range(3):
                            if fillers:
                                fillers.popleft()()

                # --- interleaved schedule (PE is in-order) ---
                add_v_fillers(0, range(KO))    # heads 0-3 need these columns
                ets = {}
                for h in range(H):
                    ets[h] = ep.tile([P, KO, NX], kvdt, tag="exp",
                                     name=f"et{h}")
                    logits_exp_slots(h, ets[h])
                    if h >= 1:
                        add_dav_fillers(h - 1, ets[h - 1])
                    if h == 1:
                        add_v_fillers(1, range(KO))
                # drain remaining fillers, then the last head's denom/AV
                while fillers:
                    fillers.popleft()()
                add_dav_fillers(H - 1, ets[H - 1])
                while fillers:
                    fillers.popleft()()

            # ========== Phase 3: LN1 -> Wo(+relu, residual) -> LN2 ==========
            # Two 512-token chunks, program-ordered so one chunk's DVE work
            # overlaps the other chunk's PE work.
            with tc.tile_pool(name="sqp", bufs=6) as sqp, \
                 tc.tile_pool(name="stp", bufs=2) as stp, \
                 tc.tile_pool(name="outp", bufs=4) as outp, \
                 tc.tile_pool(name="spp", bufs=2, space="PSUM") as spp, \
                 tc.tile_pool(name="gp3", bufs=4, space="PSUM") as pp3:

                def ln_stats(in_sb, j, lbl, sq_src=None):
                    # returns (rsig, mrs) as bf16 tiles (so the emit TTs run
                    # in the DVE's packed 2x mode) for token chunk j
                    qs = slice(j * QC, (j + 1) * QC)
                    pmu = spp.tile([P, QC], F32, tag="pmu", name=f"pmu{lbl}{j}")
                    ps2 = spp.tile([P, QC], F32, tag="ps2", name=f"ps2{lbl}{j}")
                    for do in range(KO):
                        nc.tensor.matmul(pmu, lhsT=ones_bf,
                                         rhs=in_sb[:, do, qs],
                                         start=(do == 0), stop=(do == KO - 1))
                    for do in range(KO):
                        if sq_src is not None:
                            # squares were pre-computed during attention
                            sq = sq_src[:, do, qs]
                        else:
                            sq = sqp.tile([P, QC], BF16, tag="sq",
                                          name=f"sq{lbl}{j}{do}")
                            if do % 2 == 0:
                                nc.scalar.square(sq, in_sb[:, do, qs])
                            else:
                                nc.vector.tensor_mul(sq, in_sb[:, do, qs],
                                                     in_sb[:, do, qs])
                        nc.tensor.matmul(ps2, lhsT=ones_bf, rhs=sq,
                                         start=(do == 0), stop=(do == KO - 1))
                    mu = stp.tile([P, QC], F32, tag="mu", name=f"mu{lbl}{j}")
                    nc.scalar.mul(mu, pmu, 1.0 / DIM)
                    msq = stp.tile([P, QC], F32, tag="msq", name=f"msq{lbl}{j}")
                    nc.scalar.square(msq, mu)
                    sd = stp.tile([P, QC], F32, tag="sd", name=f"sd{lbl}{j}")
                    nc.vector.scalar_tensor_tensor(
                        sd, ps2, 1.0 / DIM, msq,
                        op0=ALU.mult, op1=ALU.subtract)
                    nc.scalar.activation(sd, sd, AF.Sqrt, bias=eps_sb, scale=1.0)
                    rsf = stp.tile([P, QC], F32, tag="rsf", name=f"rsf{lbl}{j}")
                    nc.vector.reciprocal_approx_fast(rsf, sd)
                    rsig = stp.tile([P, QC], BF16, tag="rsig",
                                    name=f"rsig{lbl}{j}")
                    nc.vector.tensor_copy(rsig, rsf)
                    mrs = stp.tile([P, QC], BF16, tag="mrs", name=f"mrs{lbl}{j}")
                    nc.vector.tensor_mul(mrs, mu, rsf)
                    return rsig, mrs

                def ln_emit(in_sb, j, rsig, mrs, g_sb, b_sb, emit,
                            gpsimd_dos=()):
                    qs = slice(j * QC, (j + 1) * QC)
                    for do in range(KO):
                        t = sqp.tile([P, QC], BF16, tag="t", name=f"t{j}{do}")
                        eng = nc.gpsimd if do in gpsimd_dos else nc.vector
                        eng.tensor_mul(t, in_sb[:, do, qs], rsig)
                        eng.tensor_sub(t, t, mrs)
                        emit(do, qs, t)

                def emit_o1(do, qs, t):
                    nc.vector.tensor_scalar(
                        o1t[:, do, qs], t,
                        scalar1=g1_sb[:, do:do + 1],
                        scalar2=b1_sb[:, do:do + 1],
                        op0=ALU.mult, op1=ALU.add)

                def wo_proj(j):
                    qs = slice(j * QC, (j + 1) * QC)
                    for ng in range(2):
                        for n4 in range(4):
                            no = ng * 4 + n4
                            ps = pp3.tile([P, QC], F32, tag="ps",
                                          name=f"ps_o{j}{no}")
                            for k in range(KO):
                                nc.tensor.matmul(
                                    ps,
                                    lhsT=wo_sb[:, k, no * P:(no + 1) * P],
                                    rhs=o1t[:, k, qs],
                                    start=(k == 0), stop=(k == KO - 1))
                            ht = sqp.tile([P, QC], BF16, tag="ht",
                                          name=f"ht{j}{no}")
                            nc.scalar.activation(ht, ps, AF.Relu,
                                                 bias=bo_sb[:, no:no + 1],
                                                 scale=1.0)
                            # residual add on GpSimd (DVE is the tail's
                            # critical engine)
                            nc.gpsimd.tensor_add(z2t[:, no, qs], ht,
                                                 o1t[:, no, qs])

                def emit_o2(do, qs, t):
                    o = outp.tile([P, QC], BF16, tag="o", name=f"o{do}")
                    nc.vector.tensor_scalar(
                        o, t,
                        scalar1=g2_sb[:, do:do + 1],
                        scalar2=b2_sb[:, do:do + 1],
                        op0=ALU.mult, op1=ALU.add)
                    nc.sync.dma_start(ot3[:, do, qs], o)

                # chunk-interleaved program order for PE/DVE overlap
                s10 = ln_stats(zt, 0, "a", sq_src=sq1)
                s11 = ln_stats(zt, 1, "a", sq_src=sq1)
                ln_emit(zt, 0, *s10, g1_sb, b1_sb, emit_o1, gpsimd_dos=(6, 7))
                wo_proj(0)
                ln_emit(zt, 1, *s11, g1_sb, b1_sb, emit_o1, gpsimd_dos=(6, 7))
                s20 = ln_stats(z2t, 0, "b")
                wo_proj(1)
                ln_emit(z2t, 0, *s20, g2_sb, b2_sb, emit_o2, gpsimd_dos=(6, 7))
                s21 = ln_stats(z2t, 1, "b")
                ln_emit(z2t, 1, *s21, g2_sb, b2_sb, emit_o2, gpsimd_dos=(6, 7))

    nc.compile()
    return nc


_CACHE = {}


def _get_nc():
    if "nc" not in _CACHE:
        _CACHE["nc"] = _build()
    return _CACHE["nc"]


def make_in_maps(X, Y, mask, Wq, bq, Wk, bk, Wv, bv, Wo, bo, g1, b1, g2, b2):
    import ml_dtypes
    bf16 = ml_dtypes.bfloat16
    f8 = ml_dtypes.float8_e4m3
    kvdt = f8 if USE_FP8 else bf16
    wsc = np.float32(W_PRESCALE) if USE_FP8 else np.float32(1.0)

    f32 = lambda a: np.ascontiguousarray(np.asarray(a, dtype=np.float32))
    shared = {
        "Wq": np.ascontiguousarray(np.asarray(Wq, np.float32).astype(bf16)),
        "Wk": np.ascontiguousarray(
            (np.asarray(Wk, np.float32) * wsc).astype(kvdt)),
        "Wv": np.ascontiguousarray(
            (np.asarray(Wv, np.float32) * wsc).astype(kvdt)),
        "Wo": np.ascontiguousarray(np.asarray(Wo, np.float32).astype(bf16)),
        "bq": f32(bq), "bk": f32(bk), "bv": f32(bv), "bo": f32(bo),
        "g1": f32(g1), "b1": f32(b1), "g2": f32(g2), "b2": f32(b2),
    }
    X = np.asarray(X, dtype=np.float32)
    Y = np.asarray(Y, dtype=np.float32)
    mask = np.asarray(mask)
    in_maps = []
    for b in range(8):
        mb = np.where(mask[b], np.float32(-1e4), np.float32(0.0)).astype(np.float32)
        in_maps.append({
            "XT": np.ascontiguousarray(X[b].T.astype(bf16)),
            "YT": np.ascontiguousarray(Y[b].T.astype(kvdt)),
            "MB": mb,
            **shared,
        })
    return in_maps


def kernel(X, Y, mask, Wq, bq, Wk, bk, Wv, bv, Wo, bo, g1, b1, g2, b2,
           _trace=False):
    nc = _get_nc()
    in_maps = make_in_maps(X, Y, mask, Wq, bq, Wk, bk, Wv, bv, Wo, bo,
                           g1, b1, g2, b2)
    res = run_bass_kernel_spmd(nc, in_maps, core_ids=list(range(8)),
                               trace=_trace)
    out = np.stack([np.ascontiguousarray(
        np.asarray(res.results[b]["OT"]).astype(np.float32).T)
        for b in range(8)])
    if _trace:
        return out, res
    return out
